# revision 9
# baseline (speedup 1.0000x reference)
"""Trainium2 Bass kernel for per-token grouped attention (GQA-style).

Computation (per token t):
    q = x @ Wq.T + bq ; k = x @ Wk.T + bk ; v = x @ Wv.T + bv     (D=2048)
    reshape to (G=16 groups, d=128); scores = q_g . k_h / sqrt(d) (16x16)
    att = softmax(scores, axis=h); out = att @ v  -> (G*d,)

Sharding: data-parallel over the B*T = 16384 tokens across 8 cores
(2048 tokens/core).  Everything on-device is feature-major ("transposed")
so that the PE contracts over the partition axis; the host transposes x
on the way in and the output on the way out.

Device program (per core, SPMD):
  Phase 1 (projections): q/k use fp8-e4m3 DoubleRow matmuls (2 MACs per
    PE cell per cycle, contraction 256 per pass) -- q/k quantization
    noise is damped through the softmax so the output stays within
    tolerance; v stays bf16 (v errors pass straight to the output).
    Host pre-scales x by 16 and Wq/Wk by 512 to center the fp8 range;
    the 1/8192^2 descale folds into the softmax exp's scale operand.
    Bias added during the PSUM->SBUF copy (ACT), results kept in SBUF.
  Phase 2 (attention): tokens processed in blocks of 8; one 128x128
    matmul computes all 64 pairwise 16x16 score tiles of an 8-token
    block (only the 8 diagonal tiles are kept - masked softmax), then a
    block-diagonal trick turns att @ v into another 128x128 matmul after
    two PE transposes.  Output is written feature-major and transposed
    back on the host.
"""

import os
import numpy as np
import ml_dtypes

import concourse.bass as bass
import concourse.tile as tile
from concourse import bacc, mybir
from concourse.bass_utils import run_bass_kernel_spmd

F32 = mybir.dt.float32
BF16 = mybir.dt.bfloat16
FP8 = mybir.dt.float8e4
AF = mybir.ActivationFunctionType
ALU = mybir.AluOpType
PM = mybir.MatmulPerfMode

SCALE_X = 16.0     # fp8 pre-scale on x
SCALE_W = 512.0    # fp8 pre-scale on Wq / Wk
EXP_SCALE = 1.0 / (SCALE_X * SCALE_W) ** 2   # 2^-26, exact in f32

P = 128          # SBUF partitions
D = 2048         # model dim
G = 16           # groups
DG = 128         # per-group dim
N_CORES = 8
TC = 2048        # tokens per core
NCHUNK = 4      # phase-1 token chunks
CH = TC // NCHUNK          # 512
NTILE = 8       # phase-2 token tiles
TT = TC // NTILE           # 256
NSB = TT // 32  # super-blocks per tile (4 blocks of 8 tokens each) = 8
KT = D // P      # 16 contraction tiles
MT = D // P      # 16 output-feature tiles


def _emit(nc, tc, ctx):
    # ---- DRAM I/O -------------------------------------------------------
    # x: host pre-arranged [P, NCHUNK, KT, CH] so each chunk load is one
    # contiguous run per partition. bf16 copy feeds v; fp8 copy feeds q/k.
    xT = nc.dram_tensor("xT", [P, NCHUNK, KT, CH], BF16,
                        kind="ExternalInput").ap()
    x8T = nc.dram_tensor("x8T", [P, NCHUNK, KT, CH], FP8,
                         kind="ExternalInput").ap()
    # weights: host pre-arranged [P, MT, KT, P] (contiguous per (p, m))
    w8 = {
        p: nc.dram_tensor(f"w{p}8", [P, MT, KT, P], FP8,
                          kind="ExternalInput").ap()
        for p in "qk"
    }
    wvT = nc.dram_tensor("wv", [P, MT, KT, P], BF16, kind="ExternalInput").ap()
    b_dram = {
        p: nc.dram_tensor(f"b{p}", [P, G], F32, kind="ExternalInput").ap()
        for p in "qkv"
    }
    m01_dram = nc.dram_tensor("m01", [P, 4, P], F32, kind="ExternalInput").ap()
    ident_dram = nc.dram_tensor("ident", [P, P], BF16, kind="ExternalInput").ap()
    outT = nc.dram_tensor("outT", [D, TC], F32, kind="ExternalOutput").ap()

    # ---- pools ----------------------------------------------------------
    singles = ctx.enter_context(tc.tile_pool(name="singles", bufs=1))
    xpool = ctx.enter_context(tc.tile_pool(name="xpool", bufs=2))
    x8pool = ctx.enter_context(tc.tile_pool(name="x8pool", bufs=2))
    wpool = ctx.enter_context(tc.tile_pool(name="wpool", bufs=2))
    w8pool = ctx.enter_context(tc.tile_pool(name="w8pool", bufs=5))
    pp_ps = ctx.enter_context(tc.tile_pool(name="pp_ps", bufs=2, space="PSUM"))
    asmp = ctx.enter_context(tc.tile_pool(name="asmp", bufs=2))

    qkvp = ctx.enter_context(tc.tile_pool(name="qkvp", bufs=2))
    otp = ctx.enter_context(tc.tile_pool(name="otp", bufs=2))
    smallp = ctx.enter_context(tc.tile_pool(name="smallp", bufs=2))
    attp = ctx.enter_context(tc.tile_pool(name="attp", bufs=2))
    trp = ctx.enter_context(tc.tile_pool(name="trp", bufs=2))
    ps_s = ctx.enter_context(tc.tile_pool(name="ps_s", bufs=2, space="PSUM"))
    ps_att = ctx.enter_context(tc.tile_pool(name="ps_att", bufs=1, space="PSUM"))
    ps_vt = ctx.enter_context(tc.tile_pool(name="ps_vt", bufs=1, space="PSUM"))
    ps_o = ctx.enter_context(tc.tile_pool(name="ps_o", bufs=2, space="PSUM"))

    # ---- constants ------------------------------------------------------
    m01_sb = singles.tile([P, 4, P], F32, tag="m01", name="m01")
    nc.sync.dma_start(out=m01_sb[:], in_=m01_dram[:])
    ident_sb = singles.tile([P, P], BF16, tag="ident", name="ident")
    nc.sync.dma_start(out=ident_sb[:], in_=ident_dram[:])
    bias_sb = {}
    for p in "qkv":
        bias_sb[p] = singles.tile([P, G], F32, tag=f"bias{p}", name=f"bias{p}")
        nc.sync.dma_start(out=bias_sb[p][:], in_=b_dram[p][:])

    # assembled q/k/v chunk tiles stay resident in SBUF (block-interleaved
    # [dd, block, g, s]); attention reads them directly - no DRAM round-trip.
    chunk_asm = {}

    # DRAM views
    outT_v = outT.rearrange("(g p) t -> p g t", p=P)       # [P, G, TC]

    # ---- attention emission pieces -------------------------------------
    # Each token tile yields: a prologue (loads), 8 A-pieces (scores MMs +
    # softmax chain) and 8 B-pieces (v-transpose + att@v + out scatter), and
    # an epilogue (store).  Pieces are pumped one-per-m-group into the
    # projection emission of the NEXT chunk so DVE/ACT softmax work hides
    # under projection matmuls and the PE never waits on it.
    def make_tile_pieces(t):
        st = {}
        c, half = t // (CH // TT), t % (CH // TT)
        nb = TT // 8

        def prologue():
            st["ot"] = otp.tile([P, G, TT], F32, tag="ot", name="ot")
            st["att"] = {}

        def piece_a(sb):
            q2f = chunk_asm[c]["q"].rearrange("p b g s -> p (b g s)")
            k2f = chunk_asm[c]["k"].rearrange("p b g s -> p (b g s)")
            s_ps = ps_s.tile([P, 4, P], F32, tag="s", name="s")
            for j in range(4):
                b = half * nb + sb * 4 + j
                sl = slice(b * P, (b + 1) * P)
                nc.tensor.matmul(s_ps[:, j, :], lhsT=q2f[:, sl], rhs=k2f[:, sl],
                                 start=True, stop=True)
            # masked softmax over the 16-wide diagonal tiles; EXP_SCALE
            # undoes the fp8 pre-scaling of q and k (8192^2)
            e = smallp.tile([P, 4, P], F32, tag="e", name="e")
            nc.scalar.activation(out=e[:], in_=s_ps[:], func=AF.Exp,
                                 scale=EXP_SCALE)
            nc.vector.tensor_tensor(out=e[:], in0=e[:], in1=m01_sb[:],
                                    op=ALU.mult)
            sums = smallp.tile([P, 4], F32, tag="sums", name="sums")
            nc.vector.tensor_reduce(out=sums[:], in_=e[:],
                                    axis=mybir.AxisListType.X, op=ALU.add)
            rs = smallp.tile([P, 4], F32, tag="rs", name="rs")
            nc.vector.reciprocal(out=rs[:], in_=sums[:])
            att = attp.tile([P, 4, P], BF16, tag="att", name="att")
            for j in range(4):
                nc.vector.tensor_scalar_mul(att[:, j, :], e[:, j, :],
                                            rs[:, j:j + 1])
            a_ps = ps_att.tile([P, 4, P], BF16, tag="a", name="a")
            for j in range(4):
                nc.tensor.transpose(a_ps[:, j, :], att[:, j, :], ident_sb[:])
            attT = trp.tile([P, 4, P], BF16, tag="attT", name="attT")
            nc.scalar.copy(out=attT[:], in_=a_ps[:])
            st["att"][sb] = attT

        def piece_b(sb):
            t0 = sb * 32
            v2f = chunk_asm[c]["v"].rearrange("p b g s -> p (b g s)")
            attT = st["att"].pop(sb)
            # transpose v blocks: [d, (s,h)] -> [(s,h), d]
            v_ps = ps_vt.tile([P, 4, P], BF16, tag="v", name="v")
            for j in range(4):
                b = half * nb + sb * 4 + j
                nc.tensor.transpose(v_ps[:, j, :], v2f[:, b * P:(b + 1) * P],
                                    ident_sb[:])
            vT = trp.tile([P, 4, P], BF16, tag="vT", name="vT")
            nc.scalar.copy(out=vT[:], in_=v_ps[:])
            # att @ v -> out^T block [d, (s,g)]
            o_ps = ps_o.tile([P, 4, P], F32, tag="o", name="o")
            for j in range(4):
                nc.tensor.matmul(o_ps[:, j, :], lhsT=vT[:, j, :],
                                 rhs=attT[:, j, :], start=True, stop=True)
            dst = st["ot"][:, :, t0:t0 + 32].rearrange("p g (j s) -> p g j s", j=4)
            src = o_ps[:].rearrange("p j (g s) -> p g j s", g=16)
            nc.vector.tensor_copy(out=dst, in_=src)

        def epilogue():
            nc.gpsimd.dma_start(out=outT_v[:, :, t * TT:(t + 1) * TT],
                                in_=st["ot"][:])

        a = [lambda sb=sb: piece_a(sb) for sb in range(NSB)]
        b = [lambda sb=sb: piece_b(sb) for sb in range(NSB)]
        return prologue, a, b, epilogue

    def chunk_pieces(c):
        """Pieces for the two token tiles computed in chunk c, B lagging A."""
        t0, t1 = 2 * c, 2 * c + 1
        p0, a0, b0, e0 = make_tile_pieces(t0)
        p1, a1, b1, e1 = make_tile_pieces(t1)
        aa = a0 + a1
        bb = b0 + b1
        out = [p0, p1]
        lag = 2
        for i in range(len(aa) + lag):
            if i < len(aa):
                out.append(aa[i])
            if i >= lag:
                out.append(bb[i - lag])
        out += [e0, e1]
        return out

    def last_chunk_pieces(c):
        """Split: [qk prologues + A pieces] pumped into this chunk's own
        v-projection; [v loads + B pieces + epilogues] drain at the end."""
        t0, t1 = 2 * c, 2 * c + 1
        p0, a0, b0, e0 = make_tile_pieces(t0)
        p1, a1, b1, e1 = make_tile_pieces(t1)
        pre = [p0, p1] + a0 + a1
        post = b0 + b1 + [e0, e1]
        return pre, post

    # ---- phase 1: projections with attention pieces pumped in ----------
    def load_x(c):
        xt = xpool.tile([P, KT, CH], BF16, tag="xt", name="xt")
        nc.sync.dma_start(out=xt[:], in_=xT[:, c])
        x8t = x8pool.tile([P, KT, CH], FP8, tag="x8t", name="x8t")
        nc.sync.dma_start(out=x8t[:], in_=x8T[:, c])
        return xt, x8t

    pending = []
    post_pieces = []
    xts = {0: load_x(0)}
    for c in range(NCHUNK):
        if c + 1 < NCHUNK:
            xts[c + 1] = load_x(c + 1)
        xt, x8t = xts.pop(c)
        for p in "qkv":
            if c == NCHUNK - 1 and p == "v":
                pre, post_pieces = last_chunk_pieces(c)
                pending.extend(pre)
            asm = asmp.tile([P, CH // 8, G, 8], BF16, tag=f"asm{p}",
                            name=f"asm{p}")
            chunk_asm.setdefault(c, {})[p] = asm
            for m in range(MT):
                ps = pp_ps.tile([P, CH], F32, tag="pp", name="pp")
                if p == "v":
                    w = wpool.tile([P, KT, P], BF16, tag="wt", name="wt")
                    nc.sync.dma_start(out=w[:], in_=wvT[:, m])
                    for k in range(KT):
                        nc.tensor.matmul(
                            ps[:],
                            lhsT=w[:, k, :],
                            rhs=xt[:, k, :],
                            start=(k == 0),
                            stop=(k == KT - 1),
                        )
                else:
                    # fp8 DoubleRow: each pass contracts 2 k-tiles (256)
                    w = w8pool.tile([P, KT, P], FP8, tag="w8t", name="w8t")
                    nc.sync.dma_start(out=w[:], in_=w8[p][:, m])
                    for a in range(KT // 2):
                        nc.tensor.matmul(
                            ps[:],
                            lhsT=w[:, 2 * a:2 * a + 2, :],
                            rhs=x8t[:, 2 * a:2 * a + 2, :],
                            start=(a == 0),
                            stop=(a == KT // 2 - 1),
                            perf_mode=PM.DoubleRow,
                        )
                # bias + cast + scatter into the interleaved layout; alternate
                # ACT/DVE so neither engine's queue delays the attention chain
                dst = asm[:, :, m, :]
                src = ps[:].rearrange("p (b s) -> p b s", s=8)
                if m % 2 == 0:
                    nc.scalar.activation(out=dst, in_=src, func=AF.Identity,
                                         bias=bias_sb[p][:, m:m + 1], scale=1.0)
                else:
                    nc.vector.tensor_scalar_add(dst, src, bias_sb[p][:, m:m + 1])
                if pending:
                    pending.pop(0)()
                    if len(pending) > 16 and m % 2 == 0:
                        pending.pop(0)()
        if c < NCHUNK - 1:
            pending.extend(chunk_pieces(c))

    # drain the last chunk's attention (B pieces + epilogues)
    for piece in pending + post_pieces:
        piece()


_PROGRAM = None


def _build():
    global _PROGRAM
    if _PROGRAM is not None:
        return _PROGRAM
    from contextlib import ExitStack

    nc = bacc.Bacc("TRN2", target_bir_lowering=False, debug=False,
                   num_devices=N_CORES)
    with tile.TileContext(nc) as tc:
        with ExitStack() as ctx:
            _emit(nc, tc, ctx)
    nc.compile()
    _PROGRAM = nc
    return nc


def _prearrange_w(wT):
    """[D, D] (contraction-major) -> [P, MT, KT, P]: contiguous per (p, m)."""
    return np.ascontiguousarray(
        wT.reshape(KT, P, MT, P).transpose(1, 2, 0, 3))


def _prearrange_x(xiT):
    """[D, TC] -> [P, NCHUNK, KT, CH]: contiguous per (p, chunk)."""
    return np.ascontiguousarray(
        xiT.reshape(KT, P, NCHUNK, CH).transpose(1, 2, 0, 3))


def _host_inputs(x, Wq, bq, Wk, bk, Wv, bv):
    """Build the per-core input maps (host-side shard + transpose + cast)."""
    scale = 1.0 / np.sqrt(DG)
    xf = np.ascontiguousarray(x.reshape(-1, D))           # [16384, D]
    assert xf.shape[0] == N_CORES * TC

    bf = ml_dtypes.bfloat16
    f8 = ml_dtypes.float8_e4m3
    ss = np.float32(SCALE_X * SCALE_W)
    shared = {
        "wq8": _prearrange_w((Wq * (scale * SCALE_W)).T.astype(f8)),
        "wk8": _prearrange_w((Wk * SCALE_W).T.astype(f8)),
        "wv": _prearrange_w(Wv.T.astype(bf)),
        "bq": np.ascontiguousarray(
            (bq * (scale * ss)).reshape(G, DG).T).astype(np.float32),
        "bk": np.ascontiguousarray(
            (bk * ss).reshape(G, DG).T).astype(np.float32),
        "bv": np.ascontiguousarray(bv.reshape(G, DG).T).astype(np.float32),
        "m01": np.ascontiguousarray(np.broadcast_to(
            np.kron(np.ones((G, G), dtype=np.float32),
                    np.eye(8, dtype=np.float32))[:, None, :],
            (P, 4, P))),
        "ident": np.eye(P, dtype=np.float32).astype(bf),
    }
    in_maps = []
    for i in range(N_CORES):
        xi = xf[i * TC:(i + 1) * TC]
        xiT = np.ascontiguousarray(xi.T)
        m = dict(shared)
        m["xT"] = _prearrange_x(xiT.astype(bf))
        m["x8T"] = _prearrange_x((xiT * SCALE_X).astype(f8))
        in_maps.append(m)
    return in_maps


last_results = None


def _install_ntff_shim():
    """Provide antenv.axon_hooks if the image lacks it (profiling only)."""
    import sys
    try:
        from antenv.axon_hooks import get_axon_ntff_profile_hook  # noqa: F401
        return
    except ImportError:
        pass
    import contextlib
    import ctypes
    import types

    so_path = "/opt/axon/libaxon_pjrt.so"
    hook = None
    if os.path.exists(so_path):
        lib = ctypes.CDLL(so_path)
        if hasattr(lib, "axon_start_nrt_profile"):
            lib.axon_start_nrt_profile.argtypes = [
                ctypes.POINTER(ctypes.c_int64), ctypes.c_size_t]
            lib.axon_start_nrt_profile.restype = ctypes.c_int64
            lib.axon_stop_nrt_profile.argtypes = [ctypes.c_char_p]
            lib.axon_stop_nrt_profile.restype = ctypes.c_int64

            @contextlib.contextmanager
            def _hook(output_dir, device_ids):
                import jax
                jax.devices()
                if device_ids:
                    ids = (ctypes.c_int64 * len(device_ids))(*device_ids)
                    rc = lib.axon_start_nrt_profile(ids, len(device_ids))
                else:
                    rc = lib.axon_start_nrt_profile(None, 0)
                if rc != 0:
                    raise RuntimeError(f"axon_start_nrt_profile rc={rc}")
                try:
                    yield
                finally:
                    n = lib.axon_stop_nrt_profile(str(output_dir).encode())
                    print(f"profile: {n} file(s) written to {output_dir}")

            hook = _hook

    mod = types.ModuleType("antenv.axon_hooks")
    mod.get_axon_ntff_profile_hook = lambda: hook
    mod.set_axon_ntff_profile_hook = lambda h: None
    import antenv
    antenv.axon_hooks = mod
    sys.modules["antenv.axon_hooks"] = mod


def kernel(**inputs):
    global last_results
    nc = _build()
    in_maps = _host_inputs(**inputs)
    trace = bool(os.environ.get("BASS_TRACE"))
    if trace:
        _install_ntff_shim()
    res = run_bass_kernel_spmd(nc, in_maps, list(range(N_CORES)), trace=trace)
    last_results = res
    x = inputs["x"]
    out = np.empty((N_CORES * TC, D), dtype=np.float32)
    for i in range(N_CORES):
        out[i * TC:(i + 1) * TC] = res.results[i]["outT"].T
    return out.reshape(x.shape)



# revision 29
# speedup vs baseline: 1.3045x; 1.3045x over previous
"""Trainium2 Bass kernel for per-token grouped attention (GQA-style).

Computation (per token t):
    q = x @ Wq.T + bq ; k = x @ Wk.T + bk ; v = x @ Wv.T + bv     (D=2048)
    reshape to (G=16 groups, d=128); scores = q_g . k_h / sqrt(d) (16x16)
    att = softmax(scores, axis=h); out = att @ v  -> (G*d,)

Sharding: data-parallel over the B*T = 16384 tokens across 8 cores
(2048 tokens/core).  Everything on-device is feature-major ("transposed")
so that the PE contracts over the partition axis; the host transposes x
on the way in and the output on the way out.

Device program (per core, SPMD):
  Phase 1 (projections): q/k use fp8-e4m3 DoubleRow matmuls (2 MACs per
    PE cell per cycle, contraction 256 per pass) -- q/k quantization
    noise is damped through the softmax so the output stays within
    tolerance; v stays bf16 (v errors pass straight to the output).
    Host pre-scales x by 16 and Wq/Wk by 512 to center the fp8 range;
    the 1/8192^2 descale folds into the softmax exp's scale operand.
    Bias added during the PSUM->SBUF copy (ACT), results kept in SBUF.
  Phase 2 (attention): tokens processed in blocks of 8; one 128x128
    matmul computes all 64 pairwise 16x16 score tiles of an 8-token
    block (only the 8 diagonal tiles are kept - masked softmax), then a
    block-diagonal trick turns att @ v into another 128x128 matmul after
    two PE transposes.  Output is written feature-major and transposed
    back on the host.
"""

import os
import numpy as np
import ml_dtypes

import concourse.bass as bass
import concourse.tile as tile
from concourse import bacc, mybir
from concourse.bass_utils import run_bass_kernel_spmd

F32 = mybir.dt.float32
BF16 = mybir.dt.bfloat16
FP8 = mybir.dt.float8e4
AF = mybir.ActivationFunctionType
ALU = mybir.AluOpType
PM = mybir.MatmulPerfMode

SCALE_X = 16.0     # fp8 pre-scale on x
SCALE_W = 512.0    # fp8 pre-scale on Wq / Wk
EXP_SCALE = 1.0 / (SCALE_X * SCALE_W) ** 2   # 2^-26, exact in f32
NDR = 6            # DoubleRow passes (2 k-tiles each) per q/k m-tile
KT8 = 2 * NDR      # k-tiles contracted in fp8; rest (KT-KT8) in bf16

P = 128          # SBUF partitions
D = 2048         # model dim
G = 16           # groups
DG = 128         # per-group dim
N_CORES = 8
TC = 2048        # tokens per core
NCHUNK = 4      # phase-1 token chunks
CH = TC // NCHUNK          # 512
NTILE = 8       # phase-2 token tiles
TT = TC // NTILE           # 256
NSB = TT // 32  # super-blocks per tile (4 blocks of 8 tokens each) = 8
KT = D // P      # 16 contraction tiles
MT = D // P      # 16 output-feature tiles


def _emit(nc, tc, ctx):
    # ---- DRAM I/O -------------------------------------------------------
    # x: host pre-arranged [P, NCHUNK, KT, CH] so each chunk load is one
    # contiguous run per partition. bf16 copy feeds v; fp8 copy feeds q/k.
    xT = nc.dram_tensor("xT", [P, NCHUNK, KT, CH], BF16,
                        kind="ExternalInput").ap()
    x8T = nc.dram_tensor("x8T", [P, NCHUNK, KT8, CH], FP8,
                         kind="ExternalInput").ap()
    # weights: host pre-arranged [P, MT, KT, P] (contiguous per (p, m));
    # q/k split: k-tiles [0, KT8) as fp8, [KT8, KT) as bf16 residual
    w8 = {
        p: nc.dram_tensor(f"w{p}8", [P, MT, KT8, P], FP8,
                          kind="ExternalInput").ap()
        for p in "qk"
    }
    wr = {
        p: nc.dram_tensor(f"w{p}r", [P, MT, KT - KT8, P], BF16,
                          kind="ExternalInput").ap()
        for p in "qk"
    }
    wvT = nc.dram_tensor("wv", [P, MT, KT, P], BF16, kind="ExternalInput").ap()
    b_dram = {
        p: nc.dram_tensor(f"b{p}", [P, G], F32, kind="ExternalInput").ap()
        for p in "qkv"
    }
    m01_dram = nc.dram_tensor("m01", [P, 4, P], F32, kind="ExternalInput").ap()
    ident_dram = nc.dram_tensor("ident", [P, P], BF16, kind="ExternalInput").ap()
    outT = nc.dram_tensor("outT", [D, TC], F32, kind="ExternalOutput").ap()

    # ---- pools ----------------------------------------------------------
    singles = ctx.enter_context(tc.tile_pool(name="singles", bufs=1))
    xpool = ctx.enter_context(tc.tile_pool(name="xpool", bufs=2))
    x8pool = ctx.enter_context(tc.tile_pool(name="x8pool", bufs=2))
    wpool = ctx.enter_context(tc.tile_pool(name="wpool", bufs=3))
    w8pool = ctx.enter_context(tc.tile_pool(name="w8pool", bufs=5))
    pp_ps = ctx.enter_context(tc.tile_pool(name="pp_ps", bufs=2, space="PSUM"))
    asmp = ctx.enter_context(tc.tile_pool(name="asmp", bufs=2))

    qkvp = ctx.enter_context(tc.tile_pool(name="qkvp", bufs=2))
    otp = ctx.enter_context(tc.tile_pool(name="otp", bufs=2))
    smallp = ctx.enter_context(tc.tile_pool(name="smallp", bufs=2))
    attp = ctx.enter_context(tc.tile_pool(name="attp", bufs=2))
    trp = ctx.enter_context(tc.tile_pool(name="trp", bufs=2))
    ps_s = ctx.enter_context(tc.tile_pool(name="ps_s", bufs=2, space="PSUM"))
    ps_att = ctx.enter_context(tc.tile_pool(name="ps_att", bufs=1, space="PSUM"))
    ps_vt = ctx.enter_context(tc.tile_pool(name="ps_vt", bufs=1, space="PSUM"))
    ps_o = ctx.enter_context(tc.tile_pool(name="ps_o", bufs=2, space="PSUM"))

    # ---- constants ------------------------------------------------------
    m01_sb = singles.tile([P, 4, P], F32, tag="m01", name="m01")
    nc.sync.dma_start(out=m01_sb[:], in_=m01_dram[:])
    ident_sb = singles.tile([P, P], BF16, tag="ident", name="ident")
    nc.sync.dma_start(out=ident_sb[:], in_=ident_dram[:])
    bias_sb = {}
    for p in "qkv":
        bias_sb[p] = singles.tile([P, G], F32, tag=f"bias{p}", name=f"bias{p}")
        nc.sync.dma_start(out=bias_sb[p][:], in_=b_dram[p][:])

    # assembled q/k/v chunk tiles stay resident in SBUF (block-interleaved
    # [dd, block, g, s]); attention reads them directly - no DRAM round-trip.
    chunk_asm = {}

    # DRAM views
    outT_v = outT.rearrange("(g p) t -> p g t", p=P)       # [P, G, TC]

    # ---- attention emission pieces -------------------------------------
    # Each token tile yields: a prologue (loads), 8 A-pieces (scores MMs +
    # softmax chain) and 8 B-pieces (v-transpose + att@v + out scatter), and
    # an epilogue (store).  Pieces are pumped one-per-m-group into the
    # projection emission of the NEXT chunk so DVE/ACT softmax work hides
    # under projection matmuls and the PE never waits on it.
    def make_tile_pieces(t):
        st = {}
        c, half = t // (CH // TT), t % (CH // TT)
        nb = TT // 8

        def prologue():
            st["ot"] = otp.tile([P, G, TT], F32, tag="ot", name="ot")
            st["att"] = {}

        def piece_a(sb):
            q2f = chunk_asm[c]["q"].rearrange("p b g s -> p (b g s)")
            k2f = chunk_asm[c]["k"].rearrange("p b g s -> p (b g s)")
            s_ps = ps_s.tile([P, 4, P], F32, tag="s", name="s")
            for j in range(4):
                b = half * nb + sb * 4 + j
                sl = slice(b * P, (b + 1) * P)
                nc.tensor.matmul(s_ps[:, j, :], lhsT=q2f[:, sl], rhs=k2f[:, sl],
                                 start=True, stop=True)
            # masked softmax over the 16-wide diagonal tiles; EXP_SCALE
            # undoes the fp8 pre-scaling of q and k (8192^2)
            e = smallp.tile([P, 4, P], F32, tag="e", name="e")
            nc.scalar.activation(out=e[:], in_=s_ps[:], func=AF.Exp,
                                 scale=EXP_SCALE)
            nc.vector.tensor_tensor(out=e[:], in0=e[:], in1=m01_sb[:],
                                    op=ALU.mult)
            sums = smallp.tile([P, 4], F32, tag="sums", name="sums")
            nc.vector.tensor_reduce(out=sums[:], in_=e[:],
                                    axis=mybir.AxisListType.X, op=ALU.add)
            rs = smallp.tile([P, 4], F32, tag="rs", name="rs")
            nc.vector.reciprocal(out=rs[:], in_=sums[:])
            att = attp.tile([P, 4, P], BF16, tag="att", name="att")
            for j in range(4):
                nc.vector.tensor_scalar_mul(att[:, j, :], e[:, j, :],
                                            rs[:, j:j + 1])
            a_ps = ps_att.tile([P, 4, P], BF16, tag="a", name="a")
            for j in range(4):
                nc.tensor.transpose(a_ps[:, j, :], att[:, j, :], ident_sb[:])
            attT = trp.tile([P, 4, P], BF16, tag="attT", name="attT")
            nc.scalar.copy(out=attT[:], in_=a_ps[:])
            st["att"][sb] = attT

        def piece_b(sb):
            t0 = sb * 32
            v2f = chunk_asm[c]["v"].rearrange("p b g s -> p (b g s)")
            attT = st["att"].pop(sb)
            # transpose v blocks: [d, (s,h)] -> [(s,h), d]
            v_ps = ps_vt.tile([P, 4, P], BF16, tag="v", name="v")
            for j in range(4):
                b = half * nb + sb * 4 + j
                nc.tensor.transpose(v_ps[:, j, :], v2f[:, b * P:(b + 1) * P],
                                    ident_sb[:])
            vT = trp.tile([P, 4, P], BF16, tag="vT", name="vT")
            nc.scalar.copy(out=vT[:], in_=v_ps[:])
            # att @ v -> out^T block [d, (s,g)]
            o_ps = ps_o.tile([P, 4, P], F32, tag="o", name="o")
            for j in range(4):
                nc.tensor.matmul(o_ps[:, j, :], lhsT=vT[:, j, :],
                                 rhs=attT[:, j, :], start=True, stop=True)
            dst = st["ot"][:, :, t0:t0 + 32].rearrange("p g (j s) -> p g j s", j=4)
            src = o_ps[:].rearrange("p j (g s) -> p g j s", g=16)
            nc.vector.tensor_copy(out=dst, in_=src)

        def epilogue(h):
            # store half the tile as soon as its 4 super-blocks are done
            sl = slice(h * (TT // 2), (h + 1) * (TT // 2))
            nc.gpsimd.dma_start(
                out=outT_v[:, :, t * TT + sl.start:t * TT + sl.stop],
                in_=st["ot"][:, :, sl])

        a = [lambda sb=sb: piece_a(sb) for sb in range(NSB)]
        b = [lambda sb=sb: piece_b(sb) for sb in range(NSB)]
        # interleave the half-tile stores right after their producing pieces
        b = b[:4] + [lambda: epilogue(0)] + b[4:] + [lambda: epilogue(1)]
        return prologue, a, b

    def chunk_pieces(c):
        """Pieces for the two token tiles computed in chunk c, B lagging A."""
        t0, t1 = 2 * c, 2 * c + 1
        p0, a0, b0 = make_tile_pieces(t0)
        p1, a1, b1 = make_tile_pieces(t1)
        aa = a0 + a1
        bb = b0 + b1
        out = [p0, p1]
        lag = 2
        for i in range(max(len(aa), len(bb) + lag)):
            if i < len(aa):
                out.append(aa[i])
            if lag <= i < len(bb) + lag:
                out.append(bb[i - lag])
        return out

    def last_chunk_pieces(c):
        """Split: [qk prologues + A pieces] pumped into this chunk's own
        v-projection; [v loads + B pieces + stores] drain at the end."""
        t0, t1 = 2 * c, 2 * c + 1
        p0, a0, b0 = make_tile_pieces(t0)
        p1, a1, b1 = make_tile_pieces(t1)
        pre = [p0, p1] + a0 + a1
        post = b0 + b1
        return pre, post

    # ---- phase 1: projections with attention pieces pumped in ----------
    # x loads ride the gpsimd DMA queue (Q0) so the latency-critical weight
    # tiles on the sync queue (Q1) never wait behind a multi-MB x transfer.
    # Order: fp8 x first (q starts immediately), bf16 hi k-tiles next (q/k
    # residual matmuls), bf16 low k-tiles last (only v needs them).
    def load_x(c):
        """Allocate next chunk's x tiles; return (tiles, sub-DMA closures).

        The closures are issued interleaved between weight-tile DMAs so a
        multi-MB x transfer never sits in front of a latency-critical
        weight tile in the shared DMA queue.  Order: fp8 x first (q/k),
        then the bf16 hi k-tiles (q/k residual), then bf16 low (v only).
        """
        xt = xpool.tile([P, KT, CH], BF16, tag="xt", name="xt")
        x8t = x8pool.tile([P, KT8, CH], FP8, tag="x8t", name="x8t")
        h8 = KT8 // 2
        subs = [
            lambda: nc.sync.dma_start(out=x8t[:, :h8, :], in_=x8T[:, c, :h8]),
            lambda: nc.sync.dma_start(out=x8t[:, h8:, :], in_=x8T[:, c, h8:]),
            lambda: nc.sync.dma_start(out=xt[:, KT8:, :], in_=xT[:, c, KT8:]),
            lambda: nc.sync.dma_start(out=xt[:, 0:4, :], in_=xT[:, c, 0:4]),
            lambda: nc.sync.dma_start(out=xt[:, 4:8, :], in_=xT[:, c, 4:8]),
            lambda: nc.sync.dma_start(out=xt[:, 8:KT8, :], in_=xT[:, c, 8:KT8]),
        ]
        return (xt, x8t), subs

    pending = []
    post_pieces = []
    xprefetch = []
    (xt0, x8t0), subs0 = load_x(0)
    for s in subs0:
        s()
    xts = {0: (xt0, x8t0)}
    for c in range(NCHUNK):
        xt, x8t = xts.pop(c)
        for p in "qkv":
            if p == "k" and c + 1 < NCHUNK and c + 1 not in xts:
                xts[c + 1], xprefetch = load_x(c + 1)
            if c == NCHUNK - 1 and p == "v":
                pre, post_pieces = last_chunk_pieces(c)
                pending.extend(pre)
            asm = asmp.tile([P, CH // 8, G, 8], BF16, tag=f"asm{p}",
                            name=f"asm{p}")
            chunk_asm.setdefault(c, {})[p] = asm
            for m in range(MT):
                ps = pp_ps.tile([P, CH], F32, tag="pp", name="pp")
                if p == "v":
                    w = wpool.tile([P, KT, P], BF16, tag="wt", name="wt")
                    nc.sync.dma_start(out=w[:], in_=wvT[:, m])
                    for k in range(KT):
                        nc.tensor.matmul(
                            ps[:],
                            lhsT=w[:, k, :],
                            rhs=xt[:, k, :],
                            start=(k == 0),
                            stop=(k == KT - 1),
                        )
                else:
                    # fp8 DoubleRow passes (2 k-tiles / 256 contraction each)
                    # + bf16 residual k-tiles for precision headroom
                    w = w8pool.tile([P, KT8, P], FP8, tag="w8t", name="w8t")
                    nc.sync.dma_start(out=w[:], in_=w8[p][:, m])
                    wb = wpool.tile([P, KT - KT8, P], BF16, tag="wrt",
                                    name="wrt")
                    nc.sync.dma_start(out=wb[:], in_=wr[p][:, m])
                    for a in range(NDR):
                        nc.tensor.matmul(
                            ps[:],
                            lhsT=w[:, 2 * a:2 * a + 2, :],
                            rhs=x8t[:, 2 * a:2 * a + 2, :],
                            start=(a == 0),
                            stop=False,
                            perf_mode=PM.DoubleRow,
                        )
                    for j in range(KT - KT8):
                        nc.tensor.matmul(
                            ps[:],
                            lhsT=wb[:, j, :],
                            rhs=xt[:, KT8 + j, :],
                            start=False,
                            stop=(j == KT - KT8 - 1),
                        )
                # bias + cast + scatter into the interleaved layout; alternate
                # ACT/DVE so neither engine's queue delays the attention chain
                dst = asm[:, :, m, :]
                src = ps[:].rearrange("p (b s) -> p b s", s=8)
                if m % 2 == 0:
                    nc.scalar.activation(out=dst, in_=src, func=AF.Identity,
                                         bias=bias_sb[p][:, m:m + 1], scale=1.0)
                else:
                    nc.vector.tensor_scalar_add(dst, src, bias_sb[p][:, m:m + 1])
                if xprefetch and m % 3 == 2:
                    xprefetch.pop(0)()
                if pending:
                    pending.pop(0)()
                    if len(pending) > 16 and m % 2 == 0:
                        pending.pop(0)()
        if c < NCHUNK - 1:
            pending.extend(chunk_pieces(c))

    # drain the last chunk's attention (B pieces + epilogues)
    for piece in pending + post_pieces:
        piece()


_PROGRAM = None


def _build():
    global _PROGRAM
    if _PROGRAM is not None:
        return _PROGRAM
    from contextlib import ExitStack

    nc = bacc.Bacc("TRN2", target_bir_lowering=False, debug=False,
                   num_devices=N_CORES)
    with tile.TileContext(nc) as tc:
        with ExitStack() as ctx:
            _emit(nc, tc, ctx)
    nc.compile()
    _PROGRAM = nc
    return nc


def _prearrange_w(wT):
    """[D, D] (contraction-major) -> [P, MT, KT, P]: contiguous per (p, m)."""
    return np.ascontiguousarray(
        wT.reshape(KT, P, MT, P).transpose(1, 2, 0, 3))


def _prearrange_x(xiT, kt=KT):
    """[D, TC] -> [P, NCHUNK, kt, CH]: contiguous per (p, chunk)."""
    return np.ascontiguousarray(
        xiT.reshape(KT, P, NCHUNK, CH)[:kt].transpose(1, 2, 0, 3))


def _host_inputs(x, Wq, bq, Wk, bk, Wv, bv):
    """Build the per-core input maps (host-side shard + transpose + cast)."""
    scale = 1.0 / np.sqrt(DG)
    xf = np.ascontiguousarray(x.reshape(-1, D))           # [16384, D]
    assert xf.shape[0] == N_CORES * TC

    bf = ml_dtypes.bfloat16
    f8 = ml_dtypes.float8_e4m3
    ss = np.float32(SCALE_X * SCALE_W)
    shared = {
        "wq8": np.ascontiguousarray(_prearrange_w(
            (Wq * (scale * SCALE_W)).T.astype(f8))[:, :, :KT8]),
        "wk8": np.ascontiguousarray(
            _prearrange_w((Wk * SCALE_W).T.astype(f8))[:, :, :KT8]),
        "wqr": np.ascontiguousarray(_prearrange_w(
            (Wq * (scale * ss)).T.astype(bf))[:, :, KT8:]),
        "wkr": np.ascontiguousarray(
            _prearrange_w((Wk * ss).T.astype(bf))[:, :, KT8:]),
        "wv": _prearrange_w(Wv.T.astype(bf)),
        "bq": np.ascontiguousarray(
            (bq * (scale * ss)).reshape(G, DG).T).astype(np.float32),
        "bk": np.ascontiguousarray(
            (bk * ss).reshape(G, DG).T).astype(np.float32),
        "bv": np.ascontiguousarray(bv.reshape(G, DG).T).astype(np.float32),
        "m01": np.ascontiguousarray(np.broadcast_to(
            np.kron(np.ones((G, G), dtype=np.float32),
                    np.eye(8, dtype=np.float32))[:, None, :],
            (P, 4, P))),
        "ident": np.eye(P, dtype=np.float32).astype(bf),
    }
    in_maps = []
    for i in range(N_CORES):
        xi = xf[i * TC:(i + 1) * TC]
        xiT = np.ascontiguousarray(xi.T)
        m = dict(shared)
        m["xT"] = _prearrange_x(xiT.astype(bf))
        m["x8T"] = _prearrange_x((xiT * SCALE_X).astype(f8), kt=KT8)
        in_maps.append(m)
    return in_maps


last_results = None


def _install_ntff_shim():
    """Provide antenv.axon_hooks if the image lacks it (profiling only)."""
    import sys
    try:
        from antenv.axon_hooks import get_axon_ntff_profile_hook  # noqa: F401
        return
    except ImportError:
        pass
    import contextlib
    import ctypes
    import types

    so_path = "/opt/axon/libaxon_pjrt.so"
    hook = None
    if os.path.exists(so_path):
        lib = ctypes.CDLL(so_path)
        if hasattr(lib, "axon_start_nrt_profile"):
            lib.axon_start_nrt_profile.argtypes = [
                ctypes.POINTER(ctypes.c_int64), ctypes.c_size_t]
            lib.axon_start_nrt_profile.restype = ctypes.c_int64
            lib.axon_stop_nrt_profile.argtypes = [ctypes.c_char_p]
            lib.axon_stop_nrt_profile.restype = ctypes.c_int64

            @contextlib.contextmanager
            def _hook(output_dir, device_ids):
                import jax
                jax.devices()
                if device_ids:
                    ids = (ctypes.c_int64 * len(device_ids))(*device_ids)
                    rc = lib.axon_start_nrt_profile(ids, len(device_ids))
                else:
                    rc = lib.axon_start_nrt_profile(None, 0)
                if rc != 0:
                    raise RuntimeError(f"axon_start_nrt_profile rc={rc}")
                try:
                    yield
                finally:
                    n = lib.axon_stop_nrt_profile(str(output_dir).encode())
                    print(f"profile: {n} file(s) written to {output_dir}")

            hook = _hook

    mod = types.ModuleType("antenv.axon_hooks")
    mod.get_axon_ntff_profile_hook = lambda: hook
    mod.set_axon_ntff_profile_hook = lambda h: None
    import antenv
    antenv.axon_hooks = mod
    sys.modules["antenv.axon_hooks"] = mod


def kernel(**inputs):
    global last_results
    nc = _build()
    in_maps = _host_inputs(**inputs)
    trace = bool(os.environ.get("BASS_TRACE"))
    if trace:
        _install_ntff_shim()
    res = run_bass_kernel_spmd(nc, in_maps, list(range(N_CORES)), trace=trace)
    last_results = res
    x = inputs["x"]
    out = np.empty((N_CORES * TC, D), dtype=np.float32)
    for i in range(N_CORES):
        out[i * TC:(i + 1) * TC] = res.results[i]["outT"].T
    return out.reshape(x.shape)

